# revision 1
# baseline (speedup 1.0000x reference)
"""GCN (3-layer) + global mean pool on 8 Trainium2 NeuronCores.

Sharding: 1024 graphs -> 8 shards of 128 graphs (contiguous node ranges,
batch is sorted). Each core owns its shard's nodes and all edges whose dst
lands in the shard. Per layer, each core computes the per-node linear table
T = dinv * (h @ W) for its own nodes, an AllGather replicates the full table,
then dma_gather pulls T[src] rows per edge (ELL slots per 128-node tile,
split by int16 chunk of the global table) and a strided DVE reduce sums each
node's slots. norm factorizes as dinv[src]*dinv[dst]: dinv[src] is folded
into the table, dinv[dst] is applied after the reduce.
"""

import numpy as np

N_NODES = 100000
N_GRAPHS = 1024
N_CORES = 8
GPC = N_GRAPHS // N_CORES  # graphs per core
F = 32                     # true feature width
PF = 64                    # padded row width (256B dma_gather elem)
CHUNK = 32768              # int16-addressable table rows per gather chunk
BN_EPS = 1e-5
MAX_SLOTS_PER_CALL = 8     # 1024-idx dma_gather cap / 128

_CACHE = {}


# --------------------------------------------------------------------------
# host-side prep: sharding, ELL layout, wrapped index image
# --------------------------------------------------------------------------

def _prep(edge_index, batch):
    src_g = edge_index[0].astype(np.int64)
    dst_g = edge_index[1].astype(np.int64)
    batch = batch.astype(np.int64)
    n = N_NODES

    # in-degree including self loop (= reference deg)
    deg = np.bincount(dst_g, minlength=n).astype(np.int64) + 1

    # node ranges per core: graphs [128c, 128c+128)
    gcounts = np.bincount(batch, minlength=N_GRAPHS)
    gends = np.cumsum(gcounts)
    st = np.zeros(N_CORES + 1, np.int64)
    for c in range(1, N_CORES + 1):
        st[c] = gends[GPC * c - 1]
    sizes = np.diff(st)
    S = (int(sizes.max()) // 128 + 1) * 128  # >=1 pad row per shard
    T = S // 128

    # per-core degree sort (desc) -> local position of each node
    loc_of = np.empty(n, np.int64)
    for c in range(N_CORES):
        j = np.arange(st[c], st[c + 1])
        order = np.argsort(-deg[j], kind="stable")
        loc_of[j[order]] = np.arange(sizes[c])
    owner = np.searchsorted(st[1:], np.arange(n), side="right")
    owner_of = lambda v: np.searchsorted(st[1:], v, side="right")
    row_of = S * owner + loc_of  # global table row of node

    # all edges incl self loops, routed to dst owner
    all_src = np.concatenate([src_g, np.arange(n)])
    all_dst = np.concatenate([dst_g, np.arange(n)])
    e_owner = owner[all_dst]
    e_srcrow = row_of[all_src]
    e_dstloc = loc_of[all_dst]

    # pad (zero) rows per chunk: any never-written-by-a-real-node row < 8S
    used = np.zeros(N_CORES * S, bool)
    used[row_of] = True
    pad_rows = np.nonzero(~used)[0]
    Zq = np.empty(4, np.int64)
    for q in range(4):
        cand = pad_rows[(pad_rows >= CHUNK * q) & (pad_rows < CHUNK * (q + 1))]
        assert len(cand) > 0, f"no pad row in chunk {q}"
        Zq[q] = cand[0]

    # per-core (tile, chunk) slot counts -> common maxima
    counts = np.zeros((N_CORES, T * 128, 4), np.int32)
    core_edges = []
    for c in range(N_CORES):
        m = e_owner == c
        sr, dl = e_srcrow[m], e_dstloc[m]
        q = sr // CHUNK
        np.add.at(counts[c], (dl, q), 1)
        core_edges.append((sr, dl, q))
    cnt_tiles = counts.reshape(N_CORES, T, 128, 4)
    stq = cnt_tiles.max(axis=(0, 2)).astype(np.int64)  # [T, 4] common
    D = stq.sum(axis=1)                                # [T] slots per tile
    qoff = np.cumsum(stq, axis=1) - stq                # [T, 4] slot offset of chunk q
    tile_off = np.concatenate([[0], np.cumsum(D)])     # [T+1]
    TOT = int(tile_off[-1])

    # gather call structure (common): per tile, list of (q, slot_off, k_slots)
    calls = []
    for t in range(T):
        cl = []
        for q in range(4):
            s = int(stq[t, q])
            so = int(qoff[t, q])
            while s > 0:
                k = min(MAX_SLOTS_PER_CALL, s)
                cl.append((q, so, k))
                so += k
                s -= k
        calls.append(cl)

    # per-core flat ELL image [128, TOT] int16 (slot-major per tile)
    imgs = []
    for c in range(N_CORES):
        sr, dl, q = core_edges[c]
        img = np.empty((128, TOT), np.int16)
        for t in range(T):
            for qq in range(4):
                img[:, tile_off[t] + qoff[t, qq] : tile_off[t] + qoff[t, qq] + stq[t, qq]] = (
                    Zq[qq] - CHUNK * qq
                )
        # rank of edge within its (dst, chunk) group
        key = dl * 4 + q
        order = np.argsort(key, kind="stable")
        sk = key[order]
        starts = np.concatenate([[0], np.nonzero(np.diff(sk))[0] + 1])
        grp_start = np.zeros(len(sk), np.int64)
        grp_start[starts] = np.arange(len(sk))[starts]
        grp_start = np.maximum.accumulate(grp_start)
        rank = np.arange(len(sk)) - grp_start
        sro, dlo, qo = sr[order], dl[order], q[order]
        t_ = dlo // 128
        p_ = dlo % 128
        slot = qoff[t_, qo] + rank
        img[p_, tile_off[t_] + slot] = (sro - CHUNK * qo).astype(np.int16)
        imgs.append(img)

    # wrap for dma_gather idx layout: per call, [128, 8k] block where
    # [pp, i] = logical[(i*16 + pp%16)]  with logical j -> (slot j//128, part j%128)
    wrap_cols = 8 * TOT
    wimgs = [np.empty((128, wrap_cols), np.int16) for _ in range(N_CORES)]
    pp = np.arange(128)[:, None]
    woff_of_call = []  # per tile: list of wrapped col offsets aligned with calls
    for t in range(T):
        woffs = []
        for (q, so, k) in calls[t]:
            woffs.append(8 * (tile_off[t] + so))
            i = np.arange(8 * k)[None, :]
            j = i * 16 + (pp % 16)
            for c in range(N_CORES):
                blk = imgs[c][:, tile_off[t] + so : tile_off[t] + so + k]
                wimgs[c][:, 8 * (tile_off[t] + so) : 8 * (tile_off[t] + so + k)] = blk[
                    j % 128, j // 128
                ]
        woff_of_call.append(woffs)

    # per-core aux arrays (local order, padded to S)
    deg_loc = np.zeros((N_CORES, S, 1), np.float32)
    bat_loc = np.full((N_CORES, S, 1), 1000.0, np.float32)
    cnt_loc = np.zeros((N_CORES, 128, 1), np.float32)
    for c in range(N_CORES):
        j = np.arange(st[c], st[c + 1])
        deg_loc[c, loc_of[j], 0] = deg[j]
        bat_loc[c, loc_of[j], 0] = batch[j] - GPC * c
        cnt_loc[c, :, 0] = gcounts[GPC * c : GPC * (c + 1)]

    return dict(
        S=S, T=T, stq=stq, D=D, tile_off=tile_off, TOT=TOT, calls=calls,
        woff_of_call=woff_of_call, wimgs=wimgs, imgs=imgs, qoff=qoff,
        deg_loc=deg_loc, bat_loc=bat_loc, cnt_loc=cnt_loc, loc_of=loc_of,
        st=st, Zq=Zq, row_of=row_of,
    )


# --------------------------------------------------------------------------
# walrus workaround: at most one sem-wait per instruction
# --------------------------------------------------------------------------

def _install_tile_patch():
    import concourse.mybir as mybir
    from concourse.tile import TileContext
    from concourse.vector_clock import ScopedClock

    if getattr(TileContext, "_wait_split_installed", False):
        return

    def split_all_waits(nc):
        for bb in nc.main_func.blocks:
            insts = list(bb.instructions)
            if not any(
                i.sync_info is not None and len(i.sync_info.on_wait) > 1
                for i in insts
            ):
                continue
            newlist = []
            tail_bb = nc.cur_bb.bb if nc.cur_bb is not None else None
            for inst in insts:
                w = list(inst.sync_info.on_wait) if inst.sync_info is not None else []
                if len(w) > 1 and inst.engine != mybir.EngineType.Unassigned:
                    extra, keep = w[:-1], w[-1:]
                    inst.sync_info.on_wait = keep
                    eng = nc.engines[inst.engine]
                    for wi in extra:
                        nop = eng.nop(nofuse=True, hint="wait_split")
                        ni = nop.ins if hasattr(nop, "ins") else nop
                        if tail_bb is not None and ni in tail_bb.instructions:
                            tail_bb.instructions.remove(ni)
                        if ni.sync_info is None:
                            ni.sync_info = mybir.SyncInfo(on_wait=[], on_update=[])
                        ni.sync_info.on_wait = [wi]
                        ni.sync_info.on_update = []
                        newlist.append(ni)
                newlist.append(inst)
            bb.instructions.clear()
            for x in newlist:
                bb.instructions.append(x)

    def _patched(self, tick_clock, wait_clock):
        drain_inst = self.nc.sync.drain()
        wait_clock.add_sem_waits(
            drain_inst.ins, ScopedClock({None: tick_clock.global_clock})
        )
        self.nc.all_engine_barrier()
        assert self.sems is not None
        popped = self.nc._tile_sem_poison_stack.pop()
        assert popped is self._sem_poison
        self.nc.clear_and_free_semaphores(list(self.sems.allocated().values()))
        self.nc.all_engine_barrier()
        split_all_waits(self.nc)

    TileContext._drain_and_barrier = _patched
    TileContext._wait_split_installed = True


# --------------------------------------------------------------------------
# device program
# --------------------------------------------------------------------------

def _build(meta, n_layers=3, do_gather=True, do_reduce=True, do_tables=True, do_ag=True):
    import concourse.bacc as bacc
    import concourse.mybir as mybir
    from concourse.tile import TileContext

    _install_tile_patch()

    S, T = meta["S"], meta["T"]
    calls, woffs, tile_off, D = (
        meta["calls"], meta["woff_of_call"], meta["tile_off"], meta["D"],
    )
    TOT = meta["TOT"]
    NT = N_CORES * S  # real table rows
    f32 = mybir.dt.float32

    nc = bacc.Bacc(None, target_bir_lowering=False)
    P_ = nc.declare_dram_parameter

    x_row = P_("x_row", [1, S], f32, isOutput=False)
    deg_p = P_("deg", [S, 1], f32, isOutput=False)
    bat_p = P_("bat", [S, 1], f32, isOutput=False)
    cnt_p = P_("cnt", [128, 1], f32, isOutput=False)
    idx_p = P_("idximg", [128, 8 * TOT], mybir.dt.int16, isOutput=False)
    w1_p = P_("W1p", [1, PF], f32, isOutput=False)
    w2_p = P_("W2p", [F, PF], f32, isOutput=False)
    w3_p = P_("W3p", [F, PF], f32, isOutput=False)
    bnsc_in = [None, P_("bnsc1", [PF, 1], f32, isOutput=False),
               P_("bnsc2", [PF, 1], f32, isOutput=False)]
    bnsh_in = [None, P_("bnsh1", [PF, 1], f32, isOutput=False),
               P_("bnsh2", [PF, 1], f32, isOutput=False)]
    b3_p = P_("b3", [PF, 1], f32, isOutput=False)
    iota_p = P_("iota", [1, 128], f32, isOutput=False)
    ones_p = P_("ones", [1, 128], f32, isOutput=False)
    out_p = P_("out", [F, 128], f32, isOutput=True)

    tloc = nc.dram_tensor("tloc", [S, PF], f32)
    tab = nc.dram_tensor("tab", [4 * CHUNK, PF], f32, addr_space="Shared")

    with TileContext(nc) as tc:
        with (
            tc.tile_pool(name="const", bufs=1) as cpool,
            tc.tile_pool(name="work", bufs=3) as wpool,
            tc.tile_pool(name="msg", bufs=2) as mpool,
            tc.tile_pool(name="psum", bufs=2, space="PSUM") as ppool,
            tc.tile_pool(name="psum1", bufs=1, space="PSUM") as ppool1,
        ):
            # ---- constants ----
            w1 = cpool.tile([1, PF], f32, tag="w1")
            nc.sync.dma_start(out=w1[:], in_=w1_p[:])
            w2 = cpool.tile([F, PF], f32, tag="w2")
            nc.sync.dma_start(out=w2[:], in_=w2_p[:])
            w3 = cpool.tile([F, PF], f32, tag="w3")
            nc.sync.dma_start(out=w3[:], in_=w3_p[:])
            bnsc = [None, None, None]
            bnsh = [None, None, None]
            for L in (1, 2):
                bnsc[L] = cpool.tile([PF, 1], f32, tag=f"bnsc{L}", name=f"bnsc{L}")
                nc.sync.dma_start(out=bnsc[L][:], in_=bnsc_in[L][:])
                bnsh[L] = cpool.tile([PF, 1], f32, tag=f"bnsh{L}", name=f"bnsh{L}")
                nc.sync.dma_start(out=bnsh[L][:], in_=bnsh_in[L][:])
            b3c = cpool.tile([PF, 1], f32, tag="b3c")
            nc.sync.dma_start(out=b3c[:], in_=b3_p[:])
            xr = cpool.tile([1, S], f32, tag="xr")
            nc.sync.dma_start(out=xr[:], in_=x_row[:])

            # giota [128,128]: every partition = 0..127 row
            io = cpool.tile([1, 128], f32, tag="io")
            nc.sync.dma_start(out=io[:], in_=iota_p[:])
            on = cpool.tile([1, 128], f32, tag="on")
            nc.sync.dma_start(out=on[:], in_=ones_p[:])
            gio_ps = ppool.tile([128, 128], f32, tag="onceps", bufs=1)
            nc.tensor.matmul(out=gio_ps[:], lhsT=on[:], rhs=io[:], start=True, stop=True)
            giota = cpool.tile([128, 128], f32, tag="giota")
            nc.vector.tensor_copy(out=giota[:], in_=gio_ps[:])

            # identity for PE transpose
            ident = cpool.tile([128, 128], f32, tag="ident")
            from concourse.masks import make_identity
            make_identity(nc, ident[:])

            # dinv per tile: [128, T]
            dinv = cpool.tile([128, T], f32, tag="dinv")
            batc = cpool.tile([128, T], f32, tag="batc")
            for t in range(T):
                dg = wpool.tile([128, 1], f32, tag="dg")
                nc.sync.dma_start(out=dg[:], in_=deg_p[128 * t : 128 * (t + 1), :])
                dmx = wpool.tile([128, 1], f32, tag="dmx")
                nc.vector.tensor_scalar_max(out=dmx[:], in0=dg[:], scalar1=1.0)
                nc.scalar.activation(out=dmx[:], in_=dmx[:],
                                     func=mybir.ActivationFunctionType.Sqrt)
                nc.vector.reciprocal(out=dmx[:], in_=dmx[:])
                nc.vector.tensor_scalar_min(out=dg[:], in0=dg[:], scalar1=1.0)
                nc.vector.tensor_tensor(out=dinv[:, t : t + 1], in0=dmx[:], in1=dg[:],
                                        op=mybir.AluOpType.mult)
                bt = wpool.tile([128, 1], f32, tag="bt")
                nc.sync.dma_start(out=bt[:], in_=bat_p[128 * t : 128 * (t + 1), :])
                nc.vector.tensor_copy(out=batc[:, t : t + 1], in_=bt[:])

            # rcnt = 1/max(cnt,1)
            rcnt = cpool.tile([128, 1], f32, tag="rcnt")
            nc.sync.dma_start(out=rcnt[:], in_=cnt_p[:])
            nc.vector.tensor_scalar_max(out=rcnt[:], in0=rcnt[:], scalar1=1.0)
            nc.vector.reciprocal(out=rcnt[:], in_=rcnt[:])

            # ---- layer-1 table: T1 = dinv * (x @ W1) ----
            for t in range(T):
                ps = ppool.tile([128, PF], f32, tag="tab_ps")
                nc.tensor.matmul(out=ps[:], lhsT=xr[0:1, 128 * t : 128 * (t + 1)],
                                 rhs=w1[:], start=True, stop=True)
                tt = wpool.tile([128, PF], f32, tag="trow")
                nc.vector.tensor_tensor(out=tt[:], in0=ps[:],
                                        in1=dinv[:, t : t + 1].to_broadcast([128, PF]),
                                        op=mybir.AluOpType.mult)
                nc.sync.dma_start(out=tloc[128 * t : 128 * (t + 1), :], in_=tt[:])

            if do_ag:
                nc.gpsimd.collective_compute(
                    "AllGather", mybir.AluOpType.bypass,
                    replica_groups=[list(range(N_CORES))],
                    ins=[tloc[:]], outs=[tab[0:NT, :]],
                )

            # ---- layers ----
            pool_ps = ppool1.tile([128, F], f32, tag="pool_ps")
            for L in list((1, 2, 3))[:n_layers]:
                for t in range(T):
                    Dt = int(D[t])
                    idx_t = wpool.tile([128, 8 * Dt], mybir.dt.int16, tag="idx")
                    nc.sync.dma_start(
                        out=idx_t[:],
                        in_=idx_p[:, 8 * tile_off[t] : 8 * (tile_off[t] + Dt)],
                    )
                    msg = mpool.tile([128, Dt, PF], f32, tag="msg", name="msg") if do_gather else None
                    for ci, (q, so, k) in enumerate(calls[t] if do_gather else []):
                        wo = woffs[t][ci] - 8 * tile_off[t]
                        nc.gpsimd.dma_gather(
                            out_ap=msg[:, so : so + k, :],
                            in_ap=tab[CHUNK * q : CHUNK * (q + 1), :],
                            idxs_ap=idx_t[:, wo : wo + 8 * k],
                            num_idxs=128 * k,
                            num_idxs_reg=128 * k,
                            elem_size=PF,
                        )
                    u = wpool.tile([128, F], f32, tag="u")
                    if not (do_reduce and do_gather):
                        nc.vector.memset(u[:], 0.0)
                    else:
                        nc.vector.tensor_reduce(
                        out=u[:],
                            in_=msg[:, :, 0:F].rearrange("p s f -> p f s"),
                            axis=mybir.AxisListType.X,
                            op=mybir.AluOpType.add,
                        )
                    v = wpool.tile([128, F], f32, tag="v")
                    nc.vector.tensor_tensor(
                        out=v[:], in0=u[:],
                        in1=dinv[:, t : t + 1].to_broadcast([128, F]),
                        op=mybir.AluOpType.mult,
                    )
                    if L < 3 and not do_tables:
                        pass
                    elif L < 3:
                        # transpose -> feature domain -> bn+relu -> next table
                        vt_ps = ppool.tile([F, 128], f32, tag="vt_ps")
                        nc.tensor.transpose(out=vt_ps[:], in_=v[:], identity=ident[:])
                        ht = wpool.tile([F, 128], f32, tag="ht")
                        nc.vector.tensor_scalar(
                            out=ht[:], in0=vt_ps[:],
                            scalar1=bnsc[L][0:F, :], scalar2=bnsh[L][0:F, :],
                            op0=mybir.AluOpType.mult, op1=mybir.AluOpType.add,
                        )
                        nc.scalar.activation(out=ht[:], in_=ht[:],
                                             func=mybir.ActivationFunctionType.Relu)
                        ps = ppool.tile([128, PF], f32, tag="tab_ps")
                        wnext = w2 if L == 1 else w3
                        nc.tensor.matmul(out=ps[:], lhsT=ht[:], rhs=wnext[:],
                                         start=True, stop=True)
                        tt = wpool.tile([128, PF], f32, tag="trow")
                        nc.vector.tensor_tensor(
                            out=tt[:], in0=ps[:],
                            in1=dinv[:, t : t + 1].to_broadcast([128, PF]),
                            op=mybir.AluOpType.mult,
                        )
                        nc.sync.dma_start(out=tloc[128 * t : 128 * (t + 1), :], in_=tt[:])
                    else:
                        # pool: P[n,g] = (bat[n]==g); pool_ps += P^T... lhsT=P
                        Pm = wpool.tile([128, 128], f32, tag="Pm")
                        nc.vector.tensor_tensor(
                            out=Pm[:], in0=batc[:, t : t + 1].to_broadcast([128, 128]),
                            in1=giota[:], op=mybir.AluOpType.is_equal,
                        )
                        nc.tensor.matmul(out=pool_ps[:], lhsT=Pm[:], rhs=v[:],
                                         start=(t == 0), stop=(t == T - 1))
                if L < 3 and do_ag:
                    nc.gpsimd.collective_compute(
                        "AllGather", mybir.AluOpType.bypass,
                        replica_groups=[list(range(N_CORES))],
                        ins=[tloc[:]], outs=[tab[0:NT, :]],
                    )

            # ---- finalize pool: /cnt, transpose, +b3 ----
            pm = wpool.tile([128, F], f32, tag="pm")
            nc.vector.tensor_tensor(out=pm[:], in0=pool_ps[:],
                                    in1=rcnt[:].to_broadcast([128, F]),
                                    op=mybir.AluOpType.mult)
            pt_ps = ppool.tile([F, 128], f32, tag="onceps", bufs=1)
            nc.tensor.transpose(out=pt_ps[:], in_=pm[:], identity=ident[:])
            ot = wpool.tile([F, 128], f32, tag="ot")
            nc.vector.tensor_scalar(out=ot[:], in0=pt_ps[:], scalar1=b3c[0:F, :],
                                    scalar2=None, op0=mybir.AluOpType.add)
            nc.sync.dma_start(out=out_p[:], in_=ot[:])

    nc.finalize()
    return nc


# --------------------------------------------------------------------------
# entry point
# --------------------------------------------------------------------------

def _make_in_maps(meta, inp):
    S = meta["S"]
    x = np.asarray(inp["x"], np.float32)

    def padF(a):  # [F] -> [PF,1]
        o = np.zeros((PF, 1), np.float32)
        o[:F, 0] = np.asarray(a, np.float32)
        return o

    def padW(w):  # [k,F] -> [k,PF]
        w = np.asarray(w, np.float32)
        o = np.zeros((w.shape[0], PF), np.float32)
        o[:, :F] = w
        return o

    # bn affine: h = relu(sc*(u'+bL) + sh) with u' = dinv*u; fold bL:
    def bn_fold(g, b_, m, v, bL):
        sc = np.asarray(g) / np.sqrt(np.asarray(v) + BN_EPS)
        sh = np.asarray(b_) - np.asarray(m) * sc + sc * np.asarray(bL)
        return padF(sc), padF(sh)

    bnsc1, bnsh1 = bn_fold(inp["bn1_g"], inp["bn1_b"], inp["bn1_m"], inp["bn1_v"], inp["b1"])
    bnsc2, bnsh2 = bn_fold(inp["bn2_g"], inp["bn2_b"], inp["bn2_m"], inp["bn2_v"], inp["b2"])

    iota = np.arange(128, dtype=np.float32)[None, :]
    ones = np.ones((1, 128), np.float32)

    in_maps = []
    st, loc_of = meta["st"], meta["loc_of"]
    for c in range(N_CORES):
        xl = np.zeros((1, S), np.float32)
        j = np.arange(st[c], st[c + 1])
        xl[0, loc_of[j]] = x[j, 0]
        in_maps.append({
            "x_row": xl,
            "deg": meta["deg_loc"][c],
            "bat": meta["bat_loc"][c],
            "cnt": meta["cnt_loc"][c],
            "idximg": meta["wimgs"][c],
            "W1p": padW(inp["W1"]), "W2p": padW(inp["W2"]), "W3p": padW(inp["W3"]),
            "bnsc1": bnsc1, "bnsh1": bnsh1,
            "bnsc2": bnsc2, "bnsh2": bnsh2,
            "b3": padF(inp["b3"]),
            "iota": iota, "ones": ones,
        })
    return in_maps


def kernel(x, edge_index, batch, W1, b1, bn1_g, bn1_b, bn1_m, bn1_v,
           W2, b2, bn2_g, bn2_b, bn2_m, bn2_v, W3, b3):
    from concourse.bass_utils import run_bass_kernel_spmd

    edge_index = np.asarray(edge_index)
    batch_np = np.asarray(batch)

    key = (edge_index.shape[1], int(edge_index[0, :8].sum()), int(batch_np[:8].sum()))
    if key not in _CACHE:
        meta = _prep(edge_index, batch_np)
        nc = _build(meta)
        _CACHE[key] = (meta, nc)
    meta, nc = _CACHE[key]

    inp = dict(x=x, W1=W1, b1=b1, bn1_g=bn1_g, bn1_b=bn1_b, bn1_m=bn1_m,
               bn1_v=bn1_v, W2=W2, b2=b2, bn2_g=bn2_g, bn2_b=bn2_b,
               bn2_m=bn2_m, bn2_v=bn2_v, W3=W3, b3=b3)
    in_maps = _make_in_maps(meta, inp)

    res = run_bass_kernel_spmd(nc, in_maps, list(range(N_CORES)))
    out = np.empty((N_GRAPHS, F), np.float32)
    for c in range(N_CORES):
        out[GPC * c : GPC * (c + 1), :] = res.results[c]["out"].T
    return out



# revision 2
# speedup vs baseline: 1.3125x; 1.3125x over previous
"""GCN (3-layer) + global mean pool on 8 Trainium2 NeuronCores — V2.

Design: 1024 graphs -> 8 shards of 128 graphs (contiguous node ranges).
Each core owns its shard's ~12.5k nodes (padded to S=12800) and all edges
whose dst is in the shard (~412k incl self-loops).

Per layer:
  1. Every core builds the FULL node table T[102400, 128]fp16 redundantly:
     T = dinv * (h @ W) padded to 256B rows (32 real fp16 feats + 96 pad).
     h comes from an AllGather of per-core hT [32, 12800] fp16 (811KB/core).
  2. Exact-packed dma_gather per (dst-tile, chunk): edges stored densely in
     stream order (slot = j//128, partition = j%128), idx = srcrow within
     the 32768-row chunk, num_idxs = round16(cnt). No ELL padding.
  3. Segment-sum on PE: per 128-edge chunk, lhsT = S [128e, 128d] fp16
     indicator (DVE is_equal of dstid column vs iota) and rhs = msg fp16
     [:, slot, 0:32]; accumulate U into PSUM per dst tile.
  4. U * dinv[dst]; layers 1-2: PE-transpose, BN+ReLU in feature-major,
     append to hT_loc; layer 3: global mean pool via indicator matmul.
"""

import numpy as np

N_NODES = 100000
N_GRAPHS = 1024
N_CORES = 8
GPC = N_GRAPHS // N_CORES
F = 32
PFH = 128                  # fp16 row width (256B)
S = 12800                  # padded shard rows (multiple of 128)
T_TILES = S // 128         # 100
NT = N_CORES * S           # 102400 global table rows
CHUNK = 32768
N_CHUNKS = (NT + CHUNK - 1) // CHUNK  # 4
BN_EPS = 1e-5
MAXI = 1024                # max num_idxs per gather call
SENT = 999.0               # dstid sentinel for pad stream positions

_CACHE = {}


# --------------------------------------------------------------------------
# host-side prep
# --------------------------------------------------------------------------

def _prep(edge_index, batch):
    src_g = edge_index[0].astype(np.int64)
    dst_g = edge_index[1].astype(np.int64)
    batch = batch.astype(np.int64)
    n = N_NODES

    deg = np.bincount(dst_g, minlength=n).astype(np.int64) + 1

    # node ranges per core (graphs [128c, 128c+128))
    gcounts = np.bincount(batch, minlength=N_GRAPHS)
    gends = np.cumsum(gcounts)
    st = np.zeros(N_CORES + 1, np.int64)
    for c in range(1, N_CORES + 1):
        st[c] = gends[GPC * c - 1]
    sizes = np.diff(st)
    assert sizes.max() <= S

    # local position: keep original order (no need to degree-sort)
    loc_of = np.empty(n, np.int64)
    for c in range(N_CORES):
        j = np.arange(st[c], st[c + 1])
        loc_of[j] = np.arange(sizes[c])
    owner = np.searchsorted(st[1:], np.arange(n), side="right")
    row_of = S * owner + loc_of

    all_src = np.concatenate([src_g, np.arange(n)])
    all_dst = np.concatenate([dst_g, np.arange(n)])
    e_owner = owner[all_dst]
    e_srcrow = row_of[all_src]
    e_dstloc = loc_of[all_dst]

    # per-core edge groups by (tile, chunk); common call/chunk structure
    # (counts must be common across cores for the shared program ->
    #  use per-(t,q) max over cores as the group size, pad with idx 0 /
    #  sentinel dst)
    per_core = []
    cnts = np.zeros((N_CORES, T_TILES, N_CHUNKS), np.int64)
    for c in range(N_CORES):
        m = e_owner == c
        sr, dl = e_srcrow[m], e_dstloc[m]
        t = dl // 128
        q = sr // CHUNK
        order = np.lexsort((sr, q, t))
        sr, dl, t, q = sr[order], dl[order], t[order], q[order]
        np.add.at(cnts[c], (t, q), 1)
        per_core.append((sr, dl, t, q))
    gcnt = cnts.max(axis=0)                       # [T, Q] group sizes
    gcnt16 = ((gcnt + 15) // 16) * 16             # round16 stream length
    # slots per group (msg tile free dim), call split
    gslots = (gcnt16 + 127) // 128
    # chunk (128-edge matmul chunk) counts per group
    gchunks = gslots.copy()
    NCH = int(gchunks.sum())                      # dstid image columns
    TOTSLOT = int(gslots.sum())

    # calls: per (t,q): list of (nidx, islot_off, icol_off)
    calls = []
    iw_total = 0
    for t in range(T_TILES):
        cl = []
        for q in range(N_CHUNKS):
            rem = int(gcnt16[t, q])
            so = 0
            while rem > 0:
                nidx = min(MAXI, rem)
                cl.append((q, so, nidx, iw_total))
                iw_total += nidx // 16
                so += nidx // 128 if nidx % 128 == 0 else (nidx + 127) // 128
                rem -= nidx
        calls.append(cl)

    # build per-core idx image [128, iw_total] int16 and dstid image
    # [128, NCH] f32
    idx_imgs = np.zeros((N_CORES, 128, iw_total), np.int16)
    dst_imgs = np.full((N_CORES, 128, NCH), SENT, np.float16)
    pp16 = np.arange(128)[:, None] % 16

    # group start offsets in the global chunk counter
    ch_off = np.zeros((T_TILES, N_CHUNKS), np.int64)
    acc = 0
    for t in range(T_TILES):
        for q in range(N_CHUNKS):
            ch_off[t, q] = acc
            acc += int(gchunks[t, q])
    assert acc == NCH

    for c in range(N_CORES):
        sr, dl, t, q = per_core[c]
        # group boundaries
        key = t * N_CHUNKS + q
        # edges are sorted by (t, q); find starts
        for tt in range(T_TILES):
            pass
        starts = np.searchsorted(key, np.arange(T_TILES * N_CHUNKS))
        ends = np.searchsorted(key, np.arange(T_TILES * N_CHUNKS), side="right")
        for tt in range(T_TILES):
            for qq in range(N_CHUNKS):
                g = tt * N_CHUNKS + qq
                a, b = starts[g], ends[g]
                cnt = b - a
                L16 = int(gcnt16[tt, qq])
                if L16 == 0:
                    continue
                stream_idx = np.zeros(L16, np.int16)
                stream_dst = np.full(L16, SENT, np.float16)
                stream_idx[:cnt] = (sr[a:b] - CHUNK * qq).astype(np.int16)
                stream_dst[:cnt] = (dl[a:b] % 128).astype(np.float16)
                # dstid image: chunk ch covers stream [128ch, 128ch+128)
                nch = int(gchunks[tt, qq])
                sd = np.full(128 * nch, SENT, np.float16)
                sd[:L16] = stream_dst
                dst_imgs[c, :, ch_off[tt, qq] : ch_off[tt, qq] + nch] = (
                    sd.reshape(nch, 128).T
                )
                # idx image per call
                pos = 0
                for (qq2, so, nidx, icol) in calls[tt]:
                    if qq2 != qq:
                        continue
                    blk = stream_idx[pos : pos + nidx]
                    w = nidx // 16
                    i = np.arange(w)[None, :]
                    jj = i * 16 + pp16  # [128, w] stream positions
                    idx_imgs[c, :, icol : icol + w] = blk[np.minimum(jj, nidx - 1)]
                    pos += nidx

    # per-core aux arrays
    deg_loc = np.zeros((N_CORES, S, 1), np.float32)
    bat_loc = np.full((N_CORES, S, 1), 1000.0, np.float32)
    cnt_loc = np.zeros((N_CORES, 128, 1), np.float32)
    for c in range(N_CORES):
        j = np.arange(st[c], st[c + 1])
        deg_loc[c, loc_of[j], 0] = deg[j]
        bat_loc[c, loc_of[j], 0] = batch[j] - GPC * c
        cnt_loc[c, :, 0] = gcounts[GPC * c : GPC * (c + 1)]

    # global dinv image [128, N_CORES*T_TILES] (tile-major): col g=c*T+t
    deg_all = np.zeros((NT,), np.float32)
    for c in range(N_CORES):
        j = np.arange(st[c], st[c + 1])
        deg_all[S * c + loc_of[j]] = deg[j]
    dinv_all = np.where(deg_all > 0, 1.0 / np.sqrt(np.maximum(deg_all, 1.0)), 0.0)
    dinv_img = dinv_all.reshape(N_CORES * T_TILES, 128).T.astype(np.float32)

    MAXCH = int(gchunks.sum(axis=1).max())
    return dict(
        st=st, loc_of=loc_of, calls=calls, gcnt=gcnt, gcnt16=gcnt16,
        gslots=gslots, gchunks=gchunks, ch_off=ch_off, NCH=NCH,
        iw_total=iw_total, idx_imgs=idx_imgs, dst_imgs=dst_imgs,
        deg_loc=deg_loc, bat_loc=bat_loc, cnt_loc=cnt_loc,
        dinv_img=dinv_img, TOTSLOT=TOTSLOT, MAXCH=MAXCH,
    )


# --------------------------------------------------------------------------
# emulator (host-side validation of the device program's data flow)
# --------------------------------------------------------------------------

def _emulate(meta, inp):
    st, loc_of = meta["st"], meta["loc_of"]
    calls, gcnt16 = meta["calls"], meta["gcnt16"]
    ch_off, gchunks = meta["ch_off"], meta["gchunks"]
    idx_imgs, dst_imgs = meta["idx_imgs"], meta["dst_imgs"]
    dinv_img = meta["dinv_img"]

    x = np.asarray(inp["x"], np.float32)
    W = [np.asarray(inp[k], np.float32) for k in ("W1", "W2", "W3")]
    sc, sh = _bn_fold_all(inp)

    # x_all rows (padded)
    xall = np.zeros((NT,), np.float32)
    for c in range(N_CORES):
        j = np.arange(st[c], st[c + 1])
        xall[S * c + loc_of[j]] = x[j, 0]
    dinv_all = np.empty((NT,), np.float32)
    for g in range(N_CORES * T_TILES):
        dinv_all[128 * g : 128 * (g + 1)] = dinv_img[:, g]

    h = xall[:, None]  # [NT, 1]
    out = np.zeros((N_GRAPHS, F), np.float32)
    bat = meta["bat_loc"][:, :, 0]
    cntg = meta["cnt_loc"][:, :, 0]

    for L in range(3):
        tab = np.zeros((NT, PFH), np.float16)
        tab[:, :F if L else F] = 0
        hw = (h.astype(np.float16).astype(np.float32) @ W[L]).astype(np.float32)
        rows = (dinv_all[:, None] * hw).astype(np.float16)
        tab[:, : rows.shape[1]] = rows
        U = np.zeros((N_CORES, S, F), np.float32)
        for c in range(N_CORES):
            for t in range(T_TILES):
                psum = np.zeros((128, F), np.float32)
                for (q, so, nidx, icol) in calls[t]:
                    w = nidx // 16
                    img = idx_imgs[c][:, icol : icol + w]
                    # unwrap stream
                    stream = np.empty(nidx, np.int64)
                    ii = np.arange(nidx)
                    stream = img[ii % 16, ii // 16].astype(np.int64)
                    msg = tab[CHUNK * q + stream][:, :F].astype(np.float32)
                    # chunks
                    base_ch = ch_off[t, q]
                    for j0 in range(0, nidx, 128):
                        ch = base_ch + (so + j0 // 128)
                        dcol = dst_imgs[c][:, ch]
                        n_e = min(128, nidx - j0)
                        Sm = (dcol[:n_e, None] ==
                              np.arange(128)[None, :]).astype(np.float32)
                        psum += Sm.T @ msg[j0 : j0 + n_e]
                U[c, 128 * t : 128 * (t + 1)] = psum
        v = U * dinv_all.reshape(N_CORES, S, 1)
        if L < 2:
            hn = np.maximum(sc[L] * v + sh[L], 0.0).astype(np.float16)
            h = hn.reshape(NT, F).astype(np.float32)
        else:
            for c in range(N_CORES):
                for g in range(128):
                    m = bat[c] == g
                    ssum = v[c][m].sum(axis=0)
                    out[128 * c + g] = ssum / max(cntg[c, g], 1.0) + np.asarray(
                        inp["b3"], np.float32)
    return out


def _bn_fold_all(inp):
    sc, sh = [], []
    for g, b_, m, vv, bL in (("bn1_g", "bn1_b", "bn1_m", "bn1_v", "b1"),
                             ("bn2_g", "bn2_b", "bn2_m", "bn2_v", "b2")):
        gg = np.asarray(inp[g], np.float32)
        s = gg / np.sqrt(np.asarray(inp[vv], np.float32) + BN_EPS)
        sc.append(s)
        sh.append(np.asarray(inp[b_], np.float32)
                  - np.asarray(inp[m], np.float32) * s
                  + s * np.asarray(inp[bL], np.float32))
    return sc, sh


# --------------------------------------------------------------------------
# tile patch (same walrus workaround as V1)
# --------------------------------------------------------------------------

def _install_tile_patch():
    import concourse.mybir as mybir
    from concourse.tile import TileContext
    from concourse.vector_clock import ScopedClock

    if getattr(TileContext, "_wait_split_installed", False):
        return

    def split_all_waits(nc):
        for bb in nc.main_func.blocks:
            insts = list(bb.instructions)
            if not any(
                i.sync_info is not None and len(i.sync_info.on_wait) > 1
                for i in insts
            ):
                continue
            newlist = []
            tail_bb = nc.cur_bb.bb if nc.cur_bb is not None else None
            for inst in insts:
                w = list(inst.sync_info.on_wait) if inst.sync_info is not None else []
                if len(w) > 1 and inst.engine != mybir.EngineType.Unassigned:
                    extra, keep = w[:-1], w[-1:]
                    inst.sync_info.on_wait = keep
                    eng = nc.engines[inst.engine]
                    for wi in extra:
                        nop = eng.nop(nofuse=True, hint="wait_split")
                        ni = nop.ins if hasattr(nop, "ins") else nop
                        if tail_bb is not None and ni in tail_bb.instructions:
                            tail_bb.instructions.remove(ni)
                        if ni.sync_info is None:
                            ni.sync_info = mybir.SyncInfo(on_wait=[], on_update=[])
                        ni.sync_info.on_wait = [wi]
                        ni.sync_info.on_update = []
                        newlist.append(ni)
                newlist.append(inst)
            bb.instructions.clear()
            for x in newlist:
                bb.instructions.append(x)

    def _patched(self, tick_clock, wait_clock):
        drain_inst = self.nc.sync.drain()
        wait_clock.add_sem_waits(
            drain_inst.ins, ScopedClock({None: tick_clock.global_clock})
        )
        self.nc.all_engine_barrier()
        assert self.sems is not None
        popped = self.nc._tile_sem_poison_stack.pop()
        assert popped is self._sem_poison
        self.nc.clear_and_free_semaphores(list(self.sems.allocated().values()))
        self.nc.all_engine_barrier()
        split_all_waits(self.nc)

    TileContext._drain_and_barrier = _patched
    TileContext._wait_split_installed = True


# --------------------------------------------------------------------------
# device program
# --------------------------------------------------------------------------

def _build(meta, do_gather=True, do_smm=True, do_tables=True, nq=4):
    import concourse.bacc as bacc
    import concourse.mybir as mybir
    from concourse.tile import TileContext

    _install_tile_patch()

    calls = meta["calls"]
    ch_off = meta["ch_off"]
    NCH = meta["NCH"]
    IW = meta["iw_total"]
    MAXCH = meta["MAXCH"]
    f32 = mybir.dt.float32
    f16 = mybir.dt.float16
    AF = mybir.ActivationFunctionType
    GT = N_CORES * T_TILES  # 800 global tiles

    nc = bacc.Bacc(None, target_bir_lowering=False, num_swdge_queues=nq)
    P_ = nc.declare_dram_parameter

    idx_p = P_("idximg", [128, IW], mybir.dt.int16, isOutput=False)
    dst_p = P_("dstimg", [128, NCH], f16, isOutput=False)
    dinv_p = P_("dinvimg", [128, GT], f32, isOutput=False)
    dinvl_p = P_("dinvl", [128, T_TILES], f32, isOutput=False)
    batc_p = P_("batc", [128, T_TILES], f16, isOutput=False)
    rcnt_p = P_("rcnt", [128, 1], f32, isOutput=False)
    iota16_p = P_("iota16", [128, MAXCH * 128], f16, isOutput=False)
    dx_p = P_("dxim", [128, GT], f32, isOutput=False)
    w1_p = P_("W1t", [1, 4 * PFH], f32, isOutput=False)  # tiled x4
    w2_p = P_("W2p", [128, PFH], f16, isOutput=False)
    w3_p = P_("W3p", [128, PFH], f16, isOutput=False)
    bnsc_in = [P_("bnsc1", [F, 1], f32, isOutput=False),
               P_("bnsc2", [F, 1], f32, isOutput=False)]
    bnsh_in = [P_("bnsh1", [F, 1], f32, isOutput=False),
               P_("bnsh2", [F, 1], f32, isOutput=False)]
    b3_p = P_("b3", [F, 1], f32, isOutput=False)
    ones_p = P_("ones", [1, 128], f32, isOutput=False)
    out_p = P_("out", [F, 128], f32, isOutput=True)

    hT_loc = nc.dram_tensor("hT_loc", [F, S], f16)
    hT_all = nc.dram_tensor("hT_all", [N_CORES * F, S], f16, addr_space="Shared")
    tab = nc.dram_tensor("tab", [NT, PFH], f16)

    with TileContext(nc) as tc:
        with (
            tc.tile_pool(name="const", bufs=1) as cpool,
            tc.tile_pool(name="work", bufs=3) as wpool,
            tc.tile_pool(name="msg", bufs=6) as mpool,
            tc.tile_pool(name="idx", bufs=3) as ipool,
            tc.tile_pool(name="smat", bufs=3) as spool,
            tc.tile_pool(name="psum", bufs=2, space="PSUM") as ppool,
            tc.tile_pool(name="psum1", bufs=1, space="PSUM") as ppool1,
        ):
            # ---- constants ----
            w1t = cpool.tile([1, 4 * PFH], f32, tag="w1t")
            nc.sync.dma_start(out=w1t[:], in_=w1_p[:])
            w2 = cpool.tile([128, PFH], f16, tag="w2")
            nc.sync.dma_start(out=w2[:], in_=w2_p[:])
            w3 = cpool.tile([128, PFH], f16, tag="w3")
            nc.sync.dma_start(out=w3[:], in_=w3_p[:])
            bnsc = [None, None]
            bnsh = [None, None]
            for L in (0, 1):
                bnsc[L] = cpool.tile([F, 1], f32, tag=f"bnsc{L}", name=f"bnsc{L}")
                nc.sync.dma_start(out=bnsc[L][:], in_=bnsc_in[L][:])
                bnsh[L] = cpool.tile([F, 1], f32, tag=f"bnsh{L}", name=f"bnsh{L}")
                nc.sync.dma_start(out=bnsh[L][:], in_=bnsh_in[L][:])
            b3c = cpool.tile([F, 1], f32, tag="b3c")
            nc.sync.dma_start(out=b3c[:], in_=b3_p[:])

            dinv_g = cpool.tile([128, GT], f32, tag="dinvg")
            nc.sync.dma_start(out=dinv_g[:], in_=dinv_p[:])
            dxim = cpool.tile([128, GT], f32, tag="dxim")
            nc.sync.dma_start(out=dxim[:], in_=dx_p[:])
            dst16 = cpool.tile([128, NCH], f16, tag="dst16")
            nc.sync.dma_start(out=dst16[:], in_=dst_p[:])
            iota16 = cpool.tile([128, MAXCH * 128], f16, tag="iota16")
            nc.sync.dma_start(out=iota16[:], in_=iota16_p[:])
            dinvl = cpool.tile([128, T_TILES], f32, tag="dinvl")
            nc.sync.dma_start(out=dinvl[:], in_=dinvl_p[:])
            batc = cpool.tile([128, T_TILES], f16, tag="batc")
            nc.sync.dma_start(out=batc[:], in_=batc_p[:])
            rcnt = cpool.tile([128, 1], f32, tag="rcnt")
            nc.sync.dma_start(out=rcnt[:], in_=rcnt_p[:])
            on = cpool.tile([1, 128], f32, tag="on")
            nc.sync.dma_start(out=on[:], in_=ones_p[:])

            ident = cpool.tile([128, 128], f32, tag="ident")
            from concourse.masks import make_identity
            make_identity(nc, ident[:])

            # w1bc4 [128, 512]: every partition = tiled W1 row (x4)
            w1_ps = ppool1.tile([128, 512], f32, tag="w1_ps")
            nc.tensor.matmul(out=w1_ps[:, 0:256], lhsT=on[:],
                             rhs=w1t[:, 0:256], start=True, stop=True)
            nc.tensor.matmul(out=w1_ps[:, 256:512], lhsT=on[:],
                             rhs=w1t[:, 256:512], start=True, stop=True)
            w1bc4 = cpool.tile([128, 512], f32, tag="w1bc4")
            nc.vector.tensor_copy(out=w1bc4[:], in_=w1_ps[:])

            # pre-zero msg pool buffers (avoid NaN garbage in unwritten lanes)
            MAXK = 8
            for _ in range(6):
                mz = mpool.tile([128, MAXK, PFH], f16, tag="msg", name="msg")
                nc.vector.memset(mz[:], 0.0)

            pool_ps = ppool1.tile([128, F], f32, tag="pool_ps")

            for L in range(3):
                # ---- build full table (4 tiles per step) ----
                if L == 0:
                    for g4 in range(GT // 4 if do_tables else 1):
                        tt4 = wpool.tile([128, 512], f16, tag="trow",
                                         name="tt4")
                        nc.vector.tensor_tensor(
                            out=tt4[:].rearrange("p (c i) -> p c i", i=128),
                            in0=w1bc4[:].rearrange("p (c i) -> p c i", i=128),
                            in1=dxim[:, 4 * g4 : 4 * g4 + 4].to_broadcast(
                                [128, 4, 128]),
                            op=mybir.AluOpType.mult)
                        nc.sync.dma_start(
                            out=tab[512 * g4 : 512 * (g4 + 1), :].rearrange(
                                "(c p) f -> p c f", c=4),
                            in_=tt4[:].rearrange("p (c f) -> p c f", f=128))
                else:
                    wnext = w2 if L == 1 else w3
                    hsb = [None, None, None]
                    for i in range(3):
                        rows = min(96, N_CORES * F - 96 * i)
                        hsb[i] = cpool.tile([96, S], f16, tag=f"hsb{i}",
                                            name=f"hsb{i}")
                        nc.sync.dma_start(
                            out=hsb[i][0:rows, :],
                            in_=hT_all[96 * i : 96 * i + rows, :])
                    for g4 in range(GT // 4 if do_tables else 1):
                        ps4 = ppool.tile([128, 512], f32, tag="tab_ps",
                                         name="ps4")
                        for j in range(4):
                            g = 4 * g4 + j
                            c, t = g // T_TILES, g % T_TILES
                            hs = hsb[c // 3]
                            po = F * (c % 3)
                            nc.tensor.matmul(
                                out=ps4[:, 128 * j : 128 * (j + 1)],
                                lhsT=hs[po : po + F, 128 * t : 128 * (t + 1)],
                                rhs=wnext[po : po + F, :],
                                start=True, stop=True)
                        tt4 = wpool.tile([128, 512], f16, tag="trow",
                                         name="tt4")
                        nc.vector.tensor_tensor(
                            out=tt4[:].rearrange("p (c i) -> p c i", i=128),
                            in0=ps4[:].rearrange("p (c i) -> p c i", i=128),
                            in1=dinv_g[:, 4 * g4 : 4 * g4 + 4].to_broadcast(
                                [128, 4, 128]),
                            op=mybir.AluOpType.mult)
                        nc.sync.dma_start(
                            out=tab[512 * g4 : 512 * (g4 + 1), :].rearrange(
                                "(c p) f -> p c f", c=4),
                            in_=tt4[:].rearrange("p (c f) -> p c f", f=128))

                # ---- per local tile: gather + segment-sum ----
                if L < 2:
                    hT_sb = cpool.tile([F, S], f16, tag="hTsb", name="hTsb")
                for t in range(T_TILES):
                    Pm4 = None
                    if L == 2 and t % 4 == 0:
                        Pm4 = spool.tile([128, 512], f16, tag="Pm4",
                                         name="Pm4")
                        nb = min(4, T_TILES - t)
                        nc.vector.tensor_tensor(
                            out=Pm4[:, 0 : 128 * nb].rearrange(
                                "p (c i) -> p c i", i=128),
                            in0=batc[:, t : t + nb].to_broadcast(
                                [128, nb, 128]),
                            in1=iota16[:, 0 : 128 * nb].rearrange(
                                "p (c i) -> p c i", i=128),
                            op=mybir.AluOpType.is_equal)
                        Pm4_of = {}
                        for jj in range(nb):
                            Pm4_of[t + jj] = (Pm4, jj)
                    if L == 2 and t % 4 != 0:
                        pass
                    vdt = f16 if L == 2 else f32
                    if not calls[t]:
                        v = wpool.tile([128, F], vdt, tag="v", name="v")
                        nc.vector.memset(v[:], 0.0)
                    else:
                        ntot = sum((nidx + 127) // 128
                                   for (_, _, nidx, _) in calls[t])
                        ch0 = ch_off[t, calls[t][0][0]]
                        Sm = spool.tile([128, MAXCH * 128], f16, tag="Sm",
                                        name="Sm")
                        if do_smm:
                            nc.vector.tensor_tensor(
                                out=Sm[:, 0 : ntot * 128].rearrange(
                                    "p (c i) -> p c i", i=128),
                                in0=dst16[:, ch0 : ch0 + ntot].to_broadcast(
                                    [128, ntot, 128]),
                                in1=iota16[:, 0 : ntot * 128].rearrange(
                                    "p (c i) -> p c i", i=128),
                                op=mybir.AluOpType.is_equal)
                        u_ps = ppool.tile([128, F], f32, tag="u_ps", bufs=2)
                        done = 0
                        icol0 = calls[t][0][3]
                        wt = sum(nidx // 16 for (_, _, nidx, _) in calls[t])
                        idx_blk = ipool.tile([128, wt], mybir.dt.int16,
                                             tag="idx", name="idx")
                        nc.sync.dma_start(out=idx_blk[:],
                                          in_=idx_p[:, icol0 : icol0 + wt])
                        for ci, (q, so, nidx, icol) in enumerate(calls[t]):
                            k = (nidx + 127) // 128
                            msg = mpool.tile([128, MAXK, PFH], f16, tag="msg",
                                             name="msg")
                            if do_gather:
                                nc.gpsimd.dma_gather(
                                    out_ap=msg[:, 0:k, :],
                                    in_ap=tab[CHUNK * q :
                                              min(CHUNK * (q + 1), NT), :],
                                    idxs_ap=idx_blk[:, icol - icol0 :
                                                    icol - icol0 + nidx // 16],
                                    num_idxs=nidx,
                                    num_idxs_reg=nidx,
                                    elem_size=PFH,
                                    queue_num=ci % nq,
                                )
                            if do_smm:
                                for j in range(k):
                                    jj = ch_off[t, q] + so + j - ch0
                                    nc.tensor.matmul(
                                        out=u_ps[:],
                                        lhsT=Sm[:, 128 * jj : 128 * (jj + 1)],
                                        rhs=msg[:, j, 0:F],
                                        start=(done == 0),
                                        stop=(done == ntot - 1))
                                    done += 1
                        v = wpool.tile([128, F], vdt, tag="v", name="v")
                        if do_smm:
                            nc.scalar.activation(
                                out=v[:], in_=u_ps[:], func=AF.Copy,
                                scale=dinvl[:, t : t + 1])
                        else:
                            nc.vector.memset(v[:], 0.0)

                    if L < 2:
                        vt_ps = ppool.tile([F, 128], f32, tag="vt_ps", bufs=2)
                        nc.tensor.transpose(out=vt_ps[:], in_=v[:],
                                            identity=ident[:])
                        nc.scalar.activation(
                            out=hT_sb[:, 128 * t : 128 * (t + 1)],
                            in_=vt_ps[:], func=AF.Relu,
                            scale=bnsc[L][:], bias=bnsh[L][:])
                    else:
                        Pm, jj = Pm4_of[t]
                        nc.tensor.matmul(
                            out=pool_ps[:],
                            lhsT=Pm[:, 128 * jj : 128 * (jj + 1)],
                            rhs=v[:], start=(t == 0), stop=(t == T_TILES - 1))

                if L < 2:
                    nc.sync.dma_start(out=hT_loc[:, :], in_=hT_sb[:])
                    nc.gpsimd.collective_compute(
                        "AllGather", mybir.AluOpType.bypass,
                        replica_groups=[list(range(N_CORES))],
                        ins=[hT_loc[:]], outs=[hT_all[:]],
                    )

            # ---- finalize pool ----
            pm = wpool.tile([128, F], f32, tag="pm")
            nc.scalar.activation(out=pm[:], in_=pool_ps[:], func=AF.Copy,
                                 scale=rcnt[:])
            pt_ps = ppool.tile([F, 128], f32, tag="vt_ps", bufs=2, name="pt")
            nc.tensor.transpose(out=pt_ps[:], in_=pm[:], identity=ident[:])
            ot = wpool.tile([F, 128], f32, tag="ot")
            nc.scalar.activation(out=ot[:], in_=pt_ps[:], func=AF.Identity,
                                 bias=b3c[:])
            nc.sync.dma_start(out=out_p[:], in_=ot[:])

    nc.finalize()
    return nc


# --------------------------------------------------------------------------
# entry point
# --------------------------------------------------------------------------

def _make_in_maps(meta, inp):
    x = np.asarray(inp["x"], np.float32)
    MAXCH = meta["MAXCH"]
    GT = N_CORES * T_TILES

    def padF(a, dt=np.float32):
        o = np.zeros((F, 1), dt)
        o[:, 0] = np.asarray(a, np.float32)
        return o

    def padW4(w):  # replicate [F, F] weight at partitions 0/32/64/96
        w = np.asarray(w, np.float32)
        o = np.zeros((128, PFH), np.float16)
        for r in range(4):
            o[F * r : F * r + w.shape[0], : w.shape[1]] = w
        return o

    sc, sh = _bn_fold_all(inp)
    ones = np.ones((1, 128), np.float32)

    w1 = np.asarray(inp["W1"], np.float32)  # [1, F]
    w1row = np.zeros((PFH,), np.float32)
    w1row[:F] = w1[0]
    w1t = np.tile(w1row, 4)[None, :]

    iota16 = np.tile(np.arange(128, dtype=np.float16), MAXCH)[None, :]
    iota16 = np.repeat(iota16, 128, axis=0)

    st, loc_of = meta["st"], meta["loc_of"]
    xall = np.zeros((N_CORES * S,), np.float32)
    for c in range(N_CORES):
        j = np.arange(st[c], st[c + 1])
        xall[S * c + loc_of[j]] = x[j, 0]
    dinv_flat = np.empty((N_CORES * S,), np.float32)
    for g in range(GT):
        dinv_flat[128 * g : 128 * (g + 1)] = meta["dinv_img"][:, g]
    dxim = (dinv_flat * xall).reshape(GT, 128).T.astype(np.float32)

    in_maps = []
    for c in range(N_CORES):
        deg = meta["deg_loc"][c, :, 0]
        dinvl = np.where(deg > 0, 1.0 / np.sqrt(np.maximum(deg, 1.0)),
                         0.0).astype(np.float32).reshape(T_TILES, 128).T
        batc = meta["bat_loc"][c, :, 0].astype(np.float16).reshape(
            T_TILES, 128).T
        rcnt = (1.0 / np.maximum(meta["cnt_loc"][c], 1.0)).astype(np.float32)
        in_maps.append({
            "dxim": dxim,
            "dinvl": np.ascontiguousarray(dinvl),
            "batc": np.ascontiguousarray(batc),
            "rcnt": rcnt,
            "iota16": iota16,
            "idximg": meta["idx_imgs"][c],
            "dstimg": meta["dst_imgs"][c],
            "dinvimg": meta["dinv_img"],
            "W1t": w1t,
            "W2p": padW4(inp["W2"]),
            "W3p": padW4(inp["W3"]),
            "bnsc1": padF(sc[0]), "bnsh1": padF(sh[0]),
            "bnsc2": padF(sc[1]), "bnsh2": padF(sh[1]),
            "b3": padF(inp["b3"]),
            "ones": ones,
        })
    return in_maps


def kernel(x, edge_index, batch, W1, b1, bn1_g, bn1_b, bn1_m, bn1_v,
           W2, b2, bn2_g, bn2_b, bn2_m, bn2_v, W3, b3):
    from concourse.bass_utils import run_bass_kernel_spmd

    edge_index = np.asarray(edge_index)
    batch_np = np.asarray(batch)

    key = (edge_index.shape[1], int(edge_index[0, :8].sum()),
           int(batch_np[:8].sum()))
    if key not in _CACHE:
        meta = _prep(edge_index, batch_np)
        nc = _build(meta)
        _CACHE[key] = (meta, nc)
    meta, nc = _CACHE[key]

    inp = dict(x=x, W1=W1, b1=b1, bn1_g=bn1_g, bn1_b=bn1_b, bn1_m=bn1_m,
               bn1_v=bn1_v, W2=W2, b2=b2, bn2_g=bn2_g, bn2_b=bn2_b,
               bn2_m=bn2_m, bn2_v=bn2_v, W3=W3, b3=b3)
    in_maps = _make_in_maps(meta, inp)

    res = run_bass_kernel_spmd(nc, in_maps, list(range(N_CORES)))
    out = np.empty((N_GRAPHS, F), np.float32)
    for c in range(N_CORES):
        out[GPC * c : GPC * (c + 1), :] = res.results[c]["out"].T
    return out


# revision 3
# speedup vs baseline: 1.3758x; 1.0482x over previous
"""GCN (3-layer) + global mean pool on 8 Trainium2 NeuronCores — V2.

Design: 1024 graphs -> 8 shards of 128 graphs (contiguous node ranges).
Each core owns its shard's ~12.5k nodes (padded to S=12800) and all edges
whose dst is in the shard (~412k incl self-loops).

Per layer:
  1. Every core builds the FULL node table T[102400, 128]fp16 redundantly:
     T = dinv * (h @ W) padded to 256B rows (32 real fp16 feats + 96 pad).
     h comes from an AllGather of per-core hT [32, 12800] fp16 (811KB/core).
  2. Exact-packed dma_gather per (dst-tile, chunk): edges stored densely in
     stream order (slot = j//128, partition = j%128), idx = srcrow within
     the 32768-row chunk, num_idxs = round16(cnt). No ELL padding.
  3. Segment-sum on PE: per 128-edge chunk, lhsT = S [128e, 128d] fp16
     indicator (DVE is_equal of dstid column vs iota) and rhs = msg fp16
     [:, slot, 0:32]; accumulate U into PSUM per dst tile.
  4. U * dinv[dst]; layers 1-2: PE-transpose, BN+ReLU in feature-major,
     append to hT_loc; layer 3: global mean pool via indicator matmul.
"""

import numpy as np

N_NODES = 100000
N_GRAPHS = 1024
N_CORES = 8
GPC = N_GRAPHS // N_CORES
F = 32
PFH = 128                  # fp16 row width (256B)
S = 12800                  # padded shard rows (multiple of 128)
T_TILES = S // 128         # 100
NT = N_CORES * S           # 102400 global table rows
CHUNK = 32768
N_CHUNKS = (NT + CHUNK - 1) // CHUNK  # 4
BN_EPS = 1e-5
MAXI = 1024                # max num_idxs per gather call
SENT = 999.0               # dstid sentinel for pad stream positions

_CACHE = {}


# --------------------------------------------------------------------------
# host-side prep
# --------------------------------------------------------------------------

def _prep(edge_index, batch):
    src_g = edge_index[0].astype(np.int64)
    dst_g = edge_index[1].astype(np.int64)
    batch = batch.astype(np.int64)
    n = N_NODES

    deg = np.bincount(dst_g, minlength=n).astype(np.int64) + 1

    # node ranges per core (graphs [128c, 128c+128))
    gcounts = np.bincount(batch, minlength=N_GRAPHS)
    gends = np.cumsum(gcounts)
    st = np.zeros(N_CORES + 1, np.int64)
    for c in range(1, N_CORES + 1):
        st[c] = gends[GPC * c - 1]
    sizes = np.diff(st)
    assert sizes.max() <= S

    # local position: keep original order (no need to degree-sort)
    loc_of = np.empty(n, np.int64)
    for c in range(N_CORES):
        j = np.arange(st[c], st[c + 1])
        loc_of[j] = np.arange(sizes[c])
    owner = np.searchsorted(st[1:], np.arange(n), side="right")
    row_of = S * owner + loc_of

    all_src = np.concatenate([src_g, np.arange(n)])
    all_dst = np.concatenate([dst_g, np.arange(n)])
    e_owner = owner[all_dst]
    e_srcrow = row_of[all_src]
    e_dstloc = loc_of[all_dst]

    # per-core edge groups by (tile, chunk); common call/chunk structure
    # (counts must be common across cores for the shared program ->
    #  use per-(t,q) max over cores as the group size, pad with idx 0 /
    #  sentinel dst)
    per_core = []
    cnts = np.zeros((N_CORES, T_TILES, N_CHUNKS), np.int64)
    for c in range(N_CORES):
        m = e_owner == c
        sr, dl = e_srcrow[m], e_dstloc[m]
        t = dl // 128
        q = sr // CHUNK
        order = np.lexsort((sr, q, t))
        sr, dl, t, q = sr[order], dl[order], t[order], q[order]
        np.add.at(cnts[c], (t, q), 1)
        per_core.append((sr, dl, t, q))
    gcnt = cnts.max(axis=0)                       # [T, Q] group sizes
    gcnt16 = ((gcnt + 63) // 64) * 64             # round64 stream length
    # slots per group (msg tile free dim), call split
    gslots = (gcnt16 + 127) // 128
    # chunk (128-edge matmul chunk) counts per group
    gchunks = gslots.copy()
    NCH = int(gchunks.sum())                      # dstid image columns
    TOTSLOT = int(gslots.sum())

    # calls: per (t,q): list of (nidx, islot_off, icol_off)
    calls = []
    iw_total = 0
    for t in range(T_TILES):
        cl = []
        for q in range(N_CHUNKS):
            rem = int(gcnt16[t, q])
            so = 0
            while rem > 0:
                nidx = min(MAXI, rem)
                cl.append((q, so, nidx, iw_total))
                iw_total += nidx // 16
                so += nidx // 128 if nidx % 128 == 0 else (nidx + 127) // 128
                rem -= nidx
        calls.append(cl)

    # build per-core idx image [128, iw_total] int16 and dstid image
    # [128, NCH] f32
    idx_imgs = np.zeros((N_CORES, 128, iw_total), np.int16)
    dst_imgs = np.full((N_CORES, 128, NCH), SENT, np.float16)
    pp16 = np.arange(128)[:, None] % 16

    # group start offsets in the global chunk counter
    ch_off = np.zeros((T_TILES, N_CHUNKS), np.int64)
    acc = 0
    for t in range(T_TILES):
        for q in range(N_CHUNKS):
            ch_off[t, q] = acc
            acc += int(gchunks[t, q])
    assert acc == NCH

    for c in range(N_CORES):
        sr, dl, t, q = per_core[c]
        # group boundaries
        key = t * N_CHUNKS + q
        # edges are sorted by (t, q); find starts
        for tt in range(T_TILES):
            pass
        starts = np.searchsorted(key, np.arange(T_TILES * N_CHUNKS))
        ends = np.searchsorted(key, np.arange(T_TILES * N_CHUNKS), side="right")
        for tt in range(T_TILES):
            for qq in range(N_CHUNKS):
                g = tt * N_CHUNKS + qq
                a, b = starts[g], ends[g]
                cnt = b - a
                L16 = int(gcnt16[tt, qq])
                if L16 == 0:
                    continue
                stream_idx = np.zeros(L16, np.int16)
                stream_dst = np.full(L16, SENT, np.float16)
                stream_idx[:cnt] = (sr[a:b] - CHUNK * qq).astype(np.int16)
                stream_dst[:cnt] = (dl[a:b] % 128).astype(np.float16)
                # dstid image: chunk ch covers stream [128ch, 128ch+128)
                nch = int(gchunks[tt, qq])
                sd = np.full(128 * nch, SENT, np.float16)
                sd[:L16] = stream_dst
                dst_imgs[c, :, ch_off[tt, qq] : ch_off[tt, qq] + nch] = (
                    sd.reshape(nch, 128).T
                )
                # idx image per call
                pos = 0
                for (qq2, so, nidx, icol) in calls[tt]:
                    if qq2 != qq:
                        continue
                    blk = stream_idx[pos : pos + nidx]
                    w = nidx // 16
                    i = np.arange(w)[None, :]
                    jj = i * 16 + pp16  # [128, w] stream positions
                    idx_imgs[c, :, icol : icol + w] = blk[np.minimum(jj, nidx - 1)]
                    pos += nidx

    # per-core aux arrays
    deg_loc = np.zeros((N_CORES, S, 1), np.float32)
    bat_loc = np.full((N_CORES, S, 1), 1000.0, np.float32)
    cnt_loc = np.zeros((N_CORES, 128, 1), np.float32)
    for c in range(N_CORES):
        j = np.arange(st[c], st[c + 1])
        deg_loc[c, loc_of[j], 0] = deg[j]
        bat_loc[c, loc_of[j], 0] = batch[j] - GPC * c
        cnt_loc[c, :, 0] = gcounts[GPC * c : GPC * (c + 1)]

    # global dinv image [128, N_CORES*T_TILES] (tile-major): col g=c*T+t
    deg_all = np.zeros((NT,), np.float32)
    for c in range(N_CORES):
        j = np.arange(st[c], st[c + 1])
        deg_all[S * c + loc_of[j]] = deg[j]
    dinv_all = np.where(deg_all > 0, 1.0 / np.sqrt(np.maximum(deg_all, 1.0)), 0.0)
    dinv_img = dinv_all.reshape(N_CORES * T_TILES, 128).T.astype(np.float32)

    MAXCH = int(gchunks.sum(axis=1).max())
    return dict(
        st=st, loc_of=loc_of, calls=calls, gcnt=gcnt, gcnt16=gcnt16,
        gslots=gslots, gchunks=gchunks, ch_off=ch_off, NCH=NCH,
        iw_total=iw_total, idx_imgs=idx_imgs, dst_imgs=dst_imgs,
        deg_loc=deg_loc, bat_loc=bat_loc, cnt_loc=cnt_loc,
        dinv_img=dinv_img, TOTSLOT=TOTSLOT, MAXCH=MAXCH,
    )


# --------------------------------------------------------------------------
# emulator (host-side validation of the device program's data flow)
# --------------------------------------------------------------------------

def _emulate(meta, inp):
    st, loc_of = meta["st"], meta["loc_of"]
    calls, gcnt16 = meta["calls"], meta["gcnt16"]
    ch_off, gchunks = meta["ch_off"], meta["gchunks"]
    idx_imgs, dst_imgs = meta["idx_imgs"], meta["dst_imgs"]
    dinv_img = meta["dinv_img"]

    x = np.asarray(inp["x"], np.float32)
    W = [np.asarray(inp[k], np.float32) for k in ("W1", "W2", "W3")]
    sc, sh = _bn_fold_all(inp)

    # x_all rows (padded)
    xall = np.zeros((NT,), np.float32)
    for c in range(N_CORES):
        j = np.arange(st[c], st[c + 1])
        xall[S * c + loc_of[j]] = x[j, 0]
    dinv_all = np.empty((NT,), np.float32)
    for g in range(N_CORES * T_TILES):
        dinv_all[128 * g : 128 * (g + 1)] = dinv_img[:, g]

    h = xall[:, None]  # [NT, 1]
    out = np.zeros((N_GRAPHS, F), np.float32)
    bat = meta["bat_loc"][:, :, 0]
    cntg = meta["cnt_loc"][:, :, 0]

    for L in range(3):
        tab = np.zeros((NT, PFH), np.float16)
        tab[:, :F if L else F] = 0
        hw = (h.astype(np.float16).astype(np.float32) @ W[L]).astype(np.float32)
        rows = (dinv_all[:, None] * hw).astype(np.float16)
        tab[:, : rows.shape[1]] = rows
        U = np.zeros((N_CORES, S, F), np.float32)
        for c in range(N_CORES):
            for t in range(T_TILES):
                psum = np.zeros((128, F), np.float32)
                for (q, so, nidx, icol) in calls[t]:
                    w = nidx // 16
                    img = idx_imgs[c][:, icol : icol + w]
                    # unwrap stream
                    stream = np.empty(nidx, np.int64)
                    ii = np.arange(nidx)
                    stream = img[ii % 16, ii // 16].astype(np.int64)
                    msg = tab[CHUNK * q + stream][:, :F].astype(np.float32)
                    # chunks
                    base_ch = ch_off[t, q]
                    for j0 in range(0, nidx, 128):
                        ch = base_ch + (so + j0 // 128)
                        dcol = dst_imgs[c][:, ch]
                        n_e = min(128, nidx - j0)
                        Sm = (dcol[:n_e, None] ==
                              np.arange(128)[None, :]).astype(np.float32)
                        psum += Sm.T @ msg[j0 : j0 + n_e]
                U[c, 128 * t : 128 * (t + 1)] = psum
        v = U * dinv_all.reshape(N_CORES, S, 1)
        if L < 2:
            hn = np.maximum(sc[L] * v + sh[L], 0.0).astype(np.float16)
            h = hn.reshape(NT, F).astype(np.float32)
        else:
            for c in range(N_CORES):
                for g in range(128):
                    m = bat[c] == g
                    ssum = v[c][m].sum(axis=0)
                    out[128 * c + g] = ssum / max(cntg[c, g], 1.0) + np.asarray(
                        inp["b3"], np.float32)
    return out


def _bn_fold_all(inp):
    sc, sh = [], []
    for g, b_, m, vv, bL in (("bn1_g", "bn1_b", "bn1_m", "bn1_v", "b1"),
                             ("bn2_g", "bn2_b", "bn2_m", "bn2_v", "b2")):
        gg = np.asarray(inp[g], np.float32)
        s = gg / np.sqrt(np.asarray(inp[vv], np.float32) + BN_EPS)
        sc.append(s)
        sh.append(np.asarray(inp[b_], np.float32)
                  - np.asarray(inp[m], np.float32) * s
                  + s * np.asarray(inp[bL], np.float32))
    return sc, sh


# --------------------------------------------------------------------------
# tile patch (same walrus workaround as V1)
# --------------------------------------------------------------------------

def _install_tile_patch():
    import concourse.mybir as mybir
    from concourse.tile import TileContext
    from concourse.vector_clock import ScopedClock

    if getattr(TileContext, "_wait_split_installed", False):
        return

    def split_all_waits(nc):
        for bb in nc.main_func.blocks:
            insts = list(bb.instructions)
            if not any(
                i.sync_info is not None and len(i.sync_info.on_wait) > 1
                for i in insts
            ):
                continue
            newlist = []
            tail_bb = nc.cur_bb.bb if nc.cur_bb is not None else None
            for inst in insts:
                w = list(inst.sync_info.on_wait) if inst.sync_info is not None else []
                if len(w) > 1 and inst.engine != mybir.EngineType.Unassigned:
                    extra, keep = w[:-1], w[-1:]
                    inst.sync_info.on_wait = keep
                    eng = nc.engines[inst.engine]
                    for wi in extra:
                        nop = eng.nop(nofuse=True, hint="wait_split")
                        ni = nop.ins if hasattr(nop, "ins") else nop
                        if tail_bb is not None and ni in tail_bb.instructions:
                            tail_bb.instructions.remove(ni)
                        if ni.sync_info is None:
                            ni.sync_info = mybir.SyncInfo(on_wait=[], on_update=[])
                        ni.sync_info.on_wait = [wi]
                        ni.sync_info.on_update = []
                        newlist.append(ni)
                newlist.append(inst)
            bb.instructions.clear()
            for x in newlist:
                bb.instructions.append(x)

    def _patched(self, tick_clock, wait_clock):
        drain_inst = self.nc.sync.drain()
        wait_clock.add_sem_waits(
            drain_inst.ins, ScopedClock({None: tick_clock.global_clock})
        )
        self.nc.all_engine_barrier()
        assert self.sems is not None
        popped = self.nc._tile_sem_poison_stack.pop()
        assert popped is self._sem_poison
        self.nc.clear_and_free_semaphores(list(self.sems.allocated().values()))
        self.nc.all_engine_barrier()
        split_all_waits(self.nc)

    TileContext._drain_and_barrier = _patched
    TileContext._wait_split_installed = True


# --------------------------------------------------------------------------
# device program
# --------------------------------------------------------------------------

def _build(meta, do_gather=True, do_smm=True, do_tables=True, nq=4):
    import concourse.bacc as bacc
    import concourse.mybir as mybir
    from concourse.tile import TileContext

    _install_tile_patch()

    calls = meta["calls"]
    ch_off = meta["ch_off"]
    NCH = meta["NCH"]
    IW = meta["iw_total"]
    MAXCH = meta["MAXCH"]
    f32 = mybir.dt.float32
    f16 = mybir.dt.float16
    AF = mybir.ActivationFunctionType
    GT = N_CORES * T_TILES  # 800 global tiles

    nc = bacc.Bacc(None, target_bir_lowering=False, num_swdge_queues=nq)
    P_ = nc.declare_dram_parameter

    idx_p = P_("idximg", [128, IW], mybir.dt.int16, isOutput=False)
    dst_p = P_("dstimg", [128, NCH], f16, isOutput=False)
    dinv_p = P_("dinvimg", [128, GT], f32, isOutput=False)
    dinvl_p = P_("dinvl", [128, T_TILES], f32, isOutput=False)
    batc_p = P_("batc", [128, T_TILES], f16, isOutput=False)
    rcnt_p = P_("rcnt", [128, 1], f32, isOutput=False)
    iota16_p = P_("iota16", [128, MAXCH * 128], f16, isOutput=False)
    dx_p = P_("dxim", [128, GT], f32, isOutput=False)
    w1_p = P_("W1t", [1, 8 * PFH], f32, isOutput=False)  # tiled x8
    w2_p = P_("W2p", [128, PFH], f16, isOutput=False)
    w3_p = P_("W3p", [128, PFH], f16, isOutput=False)
    bnsc_in = [P_("bnsc1", [F, 1], f32, isOutput=False),
               P_("bnsc2", [F, 1], f32, isOutput=False)]
    bnsh_in = [P_("bnsh1", [F, 1], f32, isOutput=False),
               P_("bnsh2", [F, 1], f32, isOutput=False)]
    b3_p = P_("b3", [F, 1], f32, isOutput=False)
    ones_p = P_("ones", [1, 128], f32, isOutput=False)
    out_p = P_("out", [F, 128], f32, isOutput=True)

    hT_loc = nc.dram_tensor("hT_loc", [F, S], f16)
    hT_all = nc.dram_tensor("hT_all", [N_CORES * F, S], f16, addr_space="Shared")
    tab = nc.dram_tensor("tab", [NT, PFH], f16)

    with TileContext(nc) as tc:
        with (
            tc.tile_pool(name="const", bufs=1) as cpool,
            tc.tile_pool(name="work", bufs=3) as wpool,
            tc.tile_pool(name="msg", bufs=6) as mpool,
            tc.tile_pool(name="idx", bufs=3) as ipool,
            tc.tile_pool(name="smat", bufs=3) as spool,
            tc.tile_pool(name="psum", bufs=2, space="PSUM") as ppool,
            tc.tile_pool(name="psum1", bufs=1, space="PSUM") as ppool1,
        ):
            # ---- constants ----
            w1t = cpool.tile([1, 8 * PFH], f32, tag="w1t")
            nc.sync.dma_start(out=w1t[:], in_=w1_p[:])
            w2 = cpool.tile([128, PFH], f16, tag="w2")
            nc.sync.dma_start(out=w2[:], in_=w2_p[:])
            w3 = cpool.tile([128, PFH], f16, tag="w3")
            nc.sync.dma_start(out=w3[:], in_=w3_p[:])
            bnsc = [None, None]
            bnsh = [None, None]
            for L in (0, 1):
                bnsc[L] = cpool.tile([F, 1], f32, tag=f"bnsc{L}", name=f"bnsc{L}")
                nc.sync.dma_start(out=bnsc[L][:], in_=bnsc_in[L][:])
                bnsh[L] = cpool.tile([F, 1], f32, tag=f"bnsh{L}", name=f"bnsh{L}")
                nc.sync.dma_start(out=bnsh[L][:], in_=bnsh_in[L][:])
            b3c = cpool.tile([F, 1], f32, tag="b3c")
            nc.sync.dma_start(out=b3c[:], in_=b3_p[:])

            dinv_g = cpool.tile([128, GT], f32, tag="dinvg")
            nc.sync.dma_start(out=dinv_g[:], in_=dinv_p[:])
            dxim = cpool.tile([128, GT], f32, tag="dxim")
            nc.sync.dma_start(out=dxim[:], in_=dx_p[:])
            dst16 = cpool.tile([128, NCH], f16, tag="dst16")
            nc.sync.dma_start(out=dst16[:], in_=dst_p[:])
            iota16 = cpool.tile([128, MAXCH * 128], f16, tag="iota16")
            nc.sync.dma_start(out=iota16[:], in_=iota16_p[:])
            dinvl = cpool.tile([128, T_TILES], f32, tag="dinvl")
            nc.sync.dma_start(out=dinvl[:], in_=dinvl_p[:])
            batc = cpool.tile([128, T_TILES], f16, tag="batc")
            nc.sync.dma_start(out=batc[:], in_=batc_p[:])
            rcnt = cpool.tile([128, 1], f32, tag="rcnt")
            nc.sync.dma_start(out=rcnt[:], in_=rcnt_p[:])
            on = cpool.tile([1, 128], f32, tag="on")
            nc.sync.dma_start(out=on[:], in_=ones_p[:])

            ident = cpool.tile([128, 128], f32, tag="ident")
            from concourse.masks import make_identity
            make_identity(nc, ident[:])

            # w1bc8 [128, 1024]: every partition = tiled W1 row (x8)
            w1bc8 = cpool.tile([128, 1024], f32, tag="w1bc8")
            for half in range(2):
                w1_ps = ppool.tile([128, 512], f32, tag="tab_ps", bufs=1,
                                   name="w1ps")
                nc.tensor.matmul(out=w1_ps[:],
                                 lhsT=on[:],
                                 rhs=w1t[:, 512 * half : 512 * (half + 1)],
                                 start=True, stop=True)
                nc.vector.tensor_copy(
                    out=w1bc8[:, 512 * half : 512 * (half + 1)], in_=w1_ps[:])

            # pre-zero msg pool buffers (avoid NaN garbage in unwritten lanes)
            MAXK = 8
            for _ in range(6):
                mz = mpool.tile([128, MAXK, PFH], f16, tag="msg", name="msg")
                nc.vector.memset(mz[:], 0.0)

            pool_ps = ppool1.tile([128, F], f32, tag="pool_ps")

            nregs = {}
            def nreg(v):
                if v not in nregs:
                    nregs[v] = nc.gpsimd.to_reg(v)
                return nregs[v]

            for L in range(3):
                idx_home = {}
                # ---- build full table (4 tiles per step) ----
                if L == 0:
                    for g8 in range(GT // 8 if do_tables else 1):
                        tt8 = wpool.tile([128, 1024], f16, tag="trow",
                                         name="tt8")
                        nc.vector.tensor_tensor(
                            out=tt8[:].rearrange("p (c i) -> p c i", i=128),
                            in0=w1bc8[:].rearrange("p (c i) -> p c i", i=128),
                            in1=dxim[:, 8 * g8 : 8 * g8 + 8].to_broadcast(
                                [128, 8, 128]),
                            op=mybir.AluOpType.mult)
                        nc.sync.dma_start(
                            out=tab[1024 * g8 : 1024 * (g8 + 1), :].rearrange(
                                "(c p) f -> p c f", c=8),
                            in_=tt8[:].rearrange("p (c f) -> p c f", f=128))
                else:
                    wnext = w2 if L == 1 else w3
                    hsb = [None, None, None]
                    for i in range(3):
                        rows = min(96, N_CORES * F - 96 * i)
                        hsb[i] = cpool.tile([96, S], f16, tag=f"hsb{i}",
                                            name=f"hsb{i}")
                        nc.sync.dma_start(
                            out=hsb[i][0:rows, :],
                            in_=hT_all[96 * i : 96 * i + rows, :])
                    for g8 in range(GT // 8 if do_tables else 1):
                        ps8a = ppool.tile([128, 512], f32, tag="tab_ps",
                                          bufs=1, name="ps8a")
                        ps8b = ppool.tile([128, 512], f32, tag="tab_ps2",
                                          bufs=1, name="ps8b")
                        for j in range(8):
                            g = 8 * g8 + j
                            c, t = g // T_TILES, g % T_TILES
                            hs = hsb[c // 3]
                            po = F * (c % 3)
                            ps = ps8a if j < 4 else ps8b
                            jj = j % 4
                            nc.tensor.matmul(
                                out=ps[:, 128 * jj : 128 * (jj + 1)],
                                lhsT=hs[po : po + F, 128 * t : 128 * (t + 1)],
                                rhs=wnext[po : po + F, :],
                                start=True, stop=True)
                        tt8 = wpool.tile([128, 1024], f16, tag="trow",
                                         name="tt8")
                        for hh, ps in ((0, ps8a), (1, ps8b)):
                            nc.vector.tensor_tensor(
                                out=tt8[:, 512 * hh : 512 * (hh + 1)].rearrange(
                                    "p (c i) -> p c i", i=128),
                                in0=ps[:].rearrange("p (c i) -> p c i", i=128),
                                in1=dinv_g[:, 8 * g8 + 4 * hh :
                                           8 * g8 + 4 * hh + 4].to_broadcast(
                                    [128, 4, 128]),
                                op=mybir.AluOpType.mult)
                        nc.sync.dma_start(
                            out=tab[1024 * g8 : 1024 * (g8 + 1), :].rearrange(
                                "(c p) f -> p c f", c=8),
                            in_=tt8[:].rearrange("p (c f) -> p c f", f=128))

                # ---- per local tile: gather + segment-sum ----
                if L < 2:
                    hT_sb = cpool.tile([F, S], f16, tag="hTsb", name="hTsb")
                for t in range(T_TILES):
                    Pm4 = None
                    if L == 2 and t % 4 == 0:
                        Pm4 = spool.tile([128, 512], f16, tag="Pm4",
                                         name="Pm4")
                        nb = min(4, T_TILES - t)
                        nc.vector.tensor_tensor(
                            out=Pm4[:, 0 : 128 * nb].rearrange(
                                "p (c i) -> p c i", i=128),
                            in0=batc[:, t : t + nb].to_broadcast(
                                [128, nb, 128]),
                            in1=iota16[:, 0 : 128 * nb].rearrange(
                                "p (c i) -> p c i", i=128),
                            op=mybir.AluOpType.is_equal)
                        Pm4_of = {}
                        for jj in range(nb):
                            Pm4_of[t + jj] = (Pm4, jj)
                    if L == 2 and t % 4 != 0:
                        pass
                    vdt = f16 if L == 2 else f32
                    if not calls[t]:
                        v = wpool.tile([128, F], vdt, tag="v", name="v")
                        nc.vector.memset(v[:], 0.0)
                    else:
                        ntot = sum((nidx + 127) // 128
                                   for (_, _, nidx, _) in calls[t])
                        ch0 = ch_off[t, calls[t][0][0]]
                        Sm = spool.tile([128, MAXCH * 128], f16, tag="Sm",
                                        name="Sm")
                        if do_smm:
                            nc.vector.tensor_tensor(
                                out=Sm[:, 0 : ntot * 128].rearrange(
                                    "p (c i) -> p c i", i=128),
                                in0=dst16[:, ch0 : ch0 + ntot].to_broadcast(
                                    [128, ntot, 128]),
                                in1=iota16[:, 0 : ntot * 128].rearrange(
                                    "p (c i) -> p c i", i=128),
                                op=mybir.AluOpType.is_equal)
                        u_ps = ppool.tile([128, F], f32, tag="u_ps", bufs=2)
                        done = 0
                        if t not in idx_home:
                            t0 = (t // 4) * 4
                            span = [tt for tt in range(t0, min(t0 + 4, T_TILES))
                                    if calls[tt]]
                            c0 = calls[span[0]][0][3]
                            cend = (calls[span[-1]][-1][3]
                                    + calls[span[-1]][-1][2] // 16)
                            blk = ipool.tile([128, cend - c0],
                                             mybir.dt.int16, tag="idx",
                                             name="idx")
                            nc.sync.dma_start(out=blk[:],
                                              in_=idx_p[:, c0:cend])
                            for tt in span:
                                idx_home[tt] = (blk, c0)
                        idx_blk, icol0 = idx_home[t]
                        for ci, (q, so, nidx, icol) in enumerate(calls[t]):
                            k = (nidx + 127) // 128
                            msg = mpool.tile([128, MAXK, PFH], f16, tag="msg",
                                             name="msg")
                            if do_gather:
                                nc.gpsimd.dma_gather(
                                    out_ap=msg[:, 0:k, :],
                                    in_ap=tab[CHUNK * q :
                                              min(CHUNK * (q + 1), NT), :],
                                    idxs_ap=idx_blk[:, icol - icol0 :
                                                    icol - icol0 + nidx // 16],
                                    num_idxs=nidx,
                                    num_idxs_reg=nreg(nidx),
                                    elem_size=PFH,
                                    queue_num=ci % nq,
                                )
                            if do_smm:
                                for j in range(k):
                                    jj = ch_off[t, q] + so + j - ch0
                                    nc.tensor.matmul(
                                        out=u_ps[:],
                                        lhsT=Sm[:, 128 * jj : 128 * (jj + 1)],
                                        rhs=msg[:, j, 0:F],
                                        start=(done == 0),
                                        stop=(done == ntot - 1))
                                    done += 1
                        v = wpool.tile([128, F], vdt, tag="v", name="v")
                        if do_smm:
                            nc.scalar.activation(
                                out=v[:], in_=u_ps[:], func=AF.Copy,
                                scale=dinvl[:, t : t + 1])
                        else:
                            nc.vector.memset(v[:], 0.0)

                    if L < 2:
                        vt_ps = ppool.tile([F, 128], f32, tag="vt_ps", bufs=2)
                        nc.tensor.transpose(out=vt_ps[:], in_=v[:],
                                            identity=ident[:])
                        nc.scalar.activation(
                            out=hT_sb[:, 128 * t : 128 * (t + 1)],
                            in_=vt_ps[:], func=AF.Relu,
                            scale=bnsc[L][:], bias=bnsh[L][:])
                    else:
                        Pm, jj = Pm4_of[t]
                        nc.tensor.matmul(
                            out=pool_ps[:],
                            lhsT=Pm[:, 128 * jj : 128 * (jj + 1)],
                            rhs=v[:], start=(t == 0), stop=(t == T_TILES - 1))

                if L < 2:
                    nc.sync.dma_start(out=hT_loc[:, :], in_=hT_sb[:])
                    nc.gpsimd.collective_compute(
                        "AllGather", mybir.AluOpType.bypass,
                        replica_groups=[list(range(N_CORES))],
                        ins=[hT_loc[:]], outs=[hT_all[:]],
                    )

            # ---- finalize pool ----
            pm = wpool.tile([128, F], f32, tag="pm")
            nc.scalar.activation(out=pm[:], in_=pool_ps[:], func=AF.Copy,
                                 scale=rcnt[:])
            pt_ps = ppool.tile([F, 128], f32, tag="vt_ps", bufs=2, name="pt")
            nc.tensor.transpose(out=pt_ps[:], in_=pm[:], identity=ident[:])
            ot = wpool.tile([F, 128], f32, tag="ot")
            nc.scalar.activation(out=ot[:], in_=pt_ps[:], func=AF.Identity,
                                 bias=b3c[:])
            nc.sync.dma_start(out=out_p[:], in_=ot[:])

    nc.finalize()
    return nc


# --------------------------------------------------------------------------
# entry point
# --------------------------------------------------------------------------

def _make_in_maps(meta, inp):
    x = np.asarray(inp["x"], np.float32)
    MAXCH = meta["MAXCH"]
    GT = N_CORES * T_TILES

    def padF(a, dt=np.float32):
        o = np.zeros((F, 1), dt)
        o[:, 0] = np.asarray(a, np.float32)
        return o

    def padW4(w):  # replicate [F, F] weight at partitions 0/32/64/96
        w = np.asarray(w, np.float32)
        o = np.zeros((128, PFH), np.float16)
        for r in range(4):
            o[F * r : F * r + w.shape[0], : w.shape[1]] = w
        return o

    sc, sh = _bn_fold_all(inp)
    ones = np.ones((1, 128), np.float32)

    w1 = np.asarray(inp["W1"], np.float32)  # [1, F]
    w1row = np.zeros((PFH,), np.float32)
    w1row[:F] = w1[0]
    w1t = np.tile(w1row, 8)[None, :]

    iota16 = np.tile(np.arange(128, dtype=np.float16), MAXCH)[None, :]
    iota16 = np.repeat(iota16, 128, axis=0)

    st, loc_of = meta["st"], meta["loc_of"]
    xall = np.zeros((N_CORES * S,), np.float32)
    for c in range(N_CORES):
        j = np.arange(st[c], st[c + 1])
        xall[S * c + loc_of[j]] = x[j, 0]
    dinv_flat = np.empty((N_CORES * S,), np.float32)
    for g in range(GT):
        dinv_flat[128 * g : 128 * (g + 1)] = meta["dinv_img"][:, g]
    dxim = (dinv_flat * xall).reshape(GT, 128).T.astype(np.float32)

    in_maps = []
    for c in range(N_CORES):
        deg = meta["deg_loc"][c, :, 0]
        dinvl = np.where(deg > 0, 1.0 / np.sqrt(np.maximum(deg, 1.0)),
                         0.0).astype(np.float32).reshape(T_TILES, 128).T
        batc = meta["bat_loc"][c, :, 0].astype(np.float16).reshape(
            T_TILES, 128).T
        rcnt = (1.0 / np.maximum(meta["cnt_loc"][c], 1.0)).astype(np.float32)
        in_maps.append({
            "dxim": dxim,
            "dinvl": np.ascontiguousarray(dinvl),
            "batc": np.ascontiguousarray(batc),
            "rcnt": rcnt,
            "iota16": iota16,
            "idximg": meta["idx_imgs"][c],
            "dstimg": meta["dst_imgs"][c],
            "dinvimg": meta["dinv_img"],
            "W1t": w1t,
            "W2p": padW4(inp["W2"]),
            "W3p": padW4(inp["W3"]),
            "bnsc1": padF(sc[0]), "bnsh1": padF(sh[0]),
            "bnsc2": padF(sc[1]), "bnsh2": padF(sh[1]),
            "b3": padF(inp["b3"]),
            "ones": ones,
        })
    return in_maps


def kernel(x, edge_index, batch, W1, b1, bn1_g, bn1_b, bn1_m, bn1_v,
           W2, b2, bn2_g, bn2_b, bn2_m, bn2_v, W3, b3):
    from concourse.bass_utils import run_bass_kernel_spmd

    edge_index = np.asarray(edge_index)
    batch_np = np.asarray(batch)

    key = (edge_index.shape[1], int(edge_index[0, :8].sum()),
           int(batch_np[:8].sum()))
    if key not in _CACHE:
        meta = _prep(edge_index, batch_np)
        nc = _build(meta)
        _CACHE[key] = (meta, nc)
    meta, nc = _CACHE[key]

    inp = dict(x=x, W1=W1, b1=b1, bn1_g=bn1_g, bn1_b=bn1_b, bn1_m=bn1_m,
               bn1_v=bn1_v, W2=W2, b2=b2, bn2_g=bn2_g, bn2_b=bn2_b,
               bn2_m=bn2_m, bn2_v=bn2_v, W3=W3, b3=b3)
    in_maps = _make_in_maps(meta, inp)

    res = run_bass_kernel_spmd(nc, in_maps, list(range(N_CORES)))
    out = np.empty((N_GRAPHS, F), np.float32)
    for c in range(N_CORES):
        out[GPC * c : GPC * (c + 1), :] = res.results[c]["out"].T
    return out
